# revision 12
# baseline (speedup 1.0000x reference)
"""Trainium2 Bass kernel for an MoE classification head.

Model (per reference):
    normed = LayerNorm(x)  (no affine; shared across gate+experts)
    gate   = softmax(normed * g_g + b_g) @ gate_w + gate_b)      [B, E]
    h_e    = GELU((normed * g_e + b_e) @ w1_e + b1_e)            [E, B, H]
    out    = sum_e gate[:, e] * (h_e @ w2_e + b2_e)              [B, C]

Strategy: data-parallel over 8 NeuronCores (batch sharded 2048 rows/core,
all parameters replicated).  The per-expert LayerNorm affine folds into
w1/b1 on the host (normed*g+b) @ w1 == normed @ (g*w1) + b@w1, same for
the gate, so the device only runs: LN stats -> normalize -> transpose ->
(per expert) mm1 -> GELU -> mm2 -> gated sum.  Matmuls run in fp32r
(full-rate on the PE at N=512) accumulating in fp32 PSUM.
"""

import numpy as np

import concourse.bacc as bacc
import concourse.mybir as mybir
from concourse import tile, masks
from concourse.bass_utils import run_bass_kernel_spmd

F32 = mybir.dt.float32
F32R = mybir.dt.float32r
AF = mybir.ActivationFunctionType
ALU = mybir.AluOpType

N_CORES = 8
B, D, H, E, C = 16384, 1024, 2048, 4, 3
BS = B // N_CORES       # 2048 rows per core
NT = BS // 128          # 16 batch tiles of 128 rows
KC = D // 128           # 8 contraction chunks over D
NBC = BS // 512         # 4 batch chunks of 512 (matmul moving dim)
NHC = H // 128          # 16 H chunks
NHALF = 2               # w1 streamed per expert in halves
HCH = NHC // NHALF      # 8 H chunks per half
HHALF = H // NHALF      # 1024
EPS = 1e-5
GELU_FUNC = AF.Gelu  # sim harness may swap (CoreSim lacks Gelu)

_NC_CACHE = {}
PHASE_LEVEL = 99  # build truncation knob for HW bisection


def _build_nc():
    nc = bacc.Bacc("TRN2", target_bir_lowering=False, debug=False,
                   enable_asserts=True, num_devices=N_CORES)
    x = nc.dram_tensor("x", [BS, D], F32, kind="ExternalInput")
    gw = nc.dram_tensor("gw", [128, KC, E], F32, kind="ExternalInput")
    gb = nc.dram_tensor("gb", [1, E], F32, kind="ExternalInput")
    w1 = nc.dram_tensor("w1", [E, 128, KC, H], F32, kind="ExternalInput")
    b1 = nc.dram_tensor("b1", [E, 128, NHC], F32, kind="ExternalInput")
    w2 = nc.dram_tensor("w2", [E, 128, NHC * C], F32, kind="ExternalInput")
    b2 = nc.dram_tensor("b2", [C, E], F32, kind="ExternalInput")
    y = nc.dram_tensor("y", [C, BS], F32, kind="ExternalOutput")

    with tile.TileContext(nc) as tc:
        with (
            tc.tile_pool(name="pers", bufs=1) as pers,
            tc.tile_pool(name="xp", bufs=3) as xp,
            tc.tile_pool(name="st", bufs=2) as st,
            tc.tile_pool(name="w1p", bufs=2) as w1p,
            tc.tile_pool(name="ep", bufs=2) as ep,
            tc.tile_pool(name="hp", bufs=4) as hp,
            tc.tile_pool(name="php", bufs=4, space="PSUM") as php,
            tc.tile_pool(name="plp", bufs=4, space="PSUM") as plp,
        ):
            # ---- persistent tiles ----
            normedT = pers.tile([128, KC, BS], F32R)   # normalized x, transposed
            gwT3 = pers.tile([128, BS], F32)  # gate weights; expert e x3 rows at partitions 32e..32e+2
            accT = pers.tile([C, BS], F32)             # gated output accumulator
            identf = pers.tile([128, 128], F32)
            gwsb = pers.tile([128, KC, E], F32R)
            gbsb = pers.tile([1, E], F32R)
            onesf = pers.tile([1, 128], F32)
            onesr = pers.tile([1, 128], F32R)
            b2sb = pers.tile([C, E], F32)
            epst = pers.tile([128, 1], F32)
            nc.vector.memset(epst[:], EPS)

            masks.make_identity(nc, identf[:])
            nc.vector.memset(onesf[:], 1.0)
            nc.vector.tensor_copy(onesr[:], onesf[:])
            nc.vector.memset(accT[:], 0.0)
            nc.sync.dma_start(gwsb[:], gw[:].bitcast(F32R))
            nc.sync.dma_start(gbsb[:], gb[:].bitcast(F32R))
            nc.sync.dma_start(b2sb[:], b2[:])

            # ---- phase 0: LayerNorm + transpose + gate softmax ----
            for ti in range(NT):
                bsl = slice(ti * 128, (ti + 1) * 128)
                xt = xp.tile([128, D], F32, tag="xt")
                nc.sync.dma_start(xt[:], x[bsl, :])

                if PHASE_LEVEL < 1:
                    continue
                sm = st.tile([128, 1], F32, tag="sm")
                nc.vector.reduce_sum(sm[:], xt[:], axis=mybir.AxisListType.X)
                sq = xp.tile([128, D], F32, tag="sq", bufs=2)
                ssq = st.tile([128, 1], F32, tag="ssq")
                nc.scalar.activation(sq[:], xt[:], AF.Square, accum_out=ssq[:])
                mu = st.tile([128, 1], F32, tag="mu")
                nc.scalar.mul(mu[:], sm[:], 1.0 / D)
                ex2 = st.tile([128, 1], F32, tag="ex2")
                nc.scalar.mul(ex2[:], ssq[:], 1.0 / D)
                mu2 = st.tile([128, 1], F32, tag="mu2")
                nc.vector.tensor_mul(mu2[:], mu[:], mu[:])
                var = st.tile([128, 1], F32, tag="var")
                nc.vector.tensor_sub(var[:], ex2[:], mu2[:])
                sd = st.tile([128, 1], F32, tag="sd")
                nc.scalar.activation(sd[:], var[:], AF.Sqrt, bias=epst[:])
                rs = st.tile([128, 1], F32, tag="rs")
                nc.vector.reciprocal(rs[:], sd[:])
                nmrs = st.tile([128, 1], F32, tag="nmrs")
                nc.vector.tensor_mul(nmrs[:], mu[:], rs[:])
                nmrsn = st.tile([128, 1], F32, tag="nmrsn")
                nc.scalar.mul(nmrsn[:], nmrs[:], -1.0)
                nrm = xp.tile([128, D], F32, tag="nrm", bufs=2)
                nc.scalar.activation(nrm[:], xt[:], AF.Identity,
                                     bias=nmrsn[:], scale=rs[:])

                if PHASE_LEVEL < 2:
                    continue
                for kc in range(KC):
                    pt = php.tile([128, 128], F32, tag="mm")
                    nc.tensor.transpose(pt[:], nrm[:, kc * 128:(kc + 1) * 128],
                                        identf[:])
                    nc.vector.tensor_copy(normedT[:, kc, bsl], pt[:])

                if PHASE_LEVEL < 3:
                    continue
                pg = php.tile([128, E], F32, tag="mm")
                for kc in range(KC):
                    nc.tensor.matmul(pg[:], normedT[:, kc, bsl], gwsb[:, kc, :],
                                     start=(kc == 0), stop=False)
                nc.tensor.matmul(pg[:], onesr[:], gbsb[:], start=False, stop=True)

                mx = st.tile([128, 1], F32, tag="mx")
                nc.vector.reduce_max(mx[:], pg[:], axis=mybir.AxisListType.X)
                nmx = st.tile([128, 1], F32, tag="nmx")
                nc.scalar.mul(nmx[:], mx[:], -1.0)
                exg = xp.tile([128, E], F32, tag="exg")
                nc.scalar.activation(exg[:], pg[:], AF.Exp, bias=nmx[:])
                sme = st.tile([128, 1], F32, tag="sme")
                nc.vector.reduce_sum(sme[:], exg[:], axis=mybir.AxisListType.X)
                rinv = st.tile([128, 1], F32, tag="rinv")
                nc.vector.reciprocal(rinv[:], sme[:])
                if PHASE_LEVEL < 4:
                    continue
                gwr = xp.tile([128, 128], F32, tag="gwr")
                nc.vector.memset(gwr[:], 0.0)
                gwr4 = gwr[:].rearrange("p (e q) -> p e q", q=32)
                for j in range(C):
                    nc.vector.tensor_scalar_mul(gwr4[:, :, j], exg[:], rinv[:])
                pgt = php.tile([128, 128], F32, tag="mm")
                nc.tensor.transpose(pgt[:], gwr[:], identf[:])
                nc.vector.tensor_copy(gwT3[:, bsl], pgt[:])

            # ---- experts ----
            for e in range(E if PHASE_LEVEL >= 5 else 0):
                b1sb = ep.tile([128, NHC], F32, tag="b1")
                nc.sync.dma_start(b1sb[:], b1[e])
                w2sb = ep.tile([128, NHC * C], F32R, tag="w2")
                nc.sync.dma_start(w2sb[:], w2[e].bitcast(F32R))

                pls = [plp.tile([C, 512], F32, tag="pl", name="pl") for _ in range(NBC)]
                for half in range(NHALF):
                    w1sb = w1p.tile([128, KC, HHALF], F32R, tag="w1")
                    nc.sync.dma_start(
                        w1sb[:],
                        w1[e, :, :, half * HHALF:(half + 1) * HHALF].bitcast(F32R))
                    for hc in range(HCH):
                        hg = half * HCH + hc
                        phs = [php.tile([128, 512], F32, tag="mm", name="ph")
                               for _ in range(NBC)]
                        for kc in range(KC):
                            for bc in range(NBC):
                                nc.tensor.matmul(
                                    phs[bc][:],
                                    w1sb[:, kc, hc * 128:(hc + 1) * 128],
                                    normedT[:, kc, bc * 512:(bc + 1) * 512],
                                    start=(kc == 0), stop=(kc == KC - 1))
                        for bc in range(NBC):
                            hT = hp.tile([128, 512], F32R, tag="hT")
                            nc.scalar.activation(hT[:], phs[bc][:], GELU_FUNC,
                                                 bias=b1sb[:, hg:hg + 1])
                            nc.tensor.matmul(
                                pls[bc][:], w2sb[:, hg * C:(hg + 1) * C], hT[:],
                                start=(hg == 0), stop=(hg == NHC - 1))

                for bc in range(NBC if PHASE_LEVEL >= 6 else 0):
                    csl = slice(bc * 512, (bc + 1) * 512)
                    gwb = hp.tile([C, 512], F32, tag="gwb", bufs=2)
                    nc.sync.dma_start(gwb[:], gwT3[32 * e:32 * e + C, csl])
                    lt = hp.tile([C, 512], F32, tag="lt", bufs=2)
                    nc.scalar.activation(lt[:], pls[bc][:], AF.Identity,
                                         bias=b2sb[:, e:e + 1])
                    nc.vector.tensor_mul(lt[:], lt[:], gwb[:])
                    nc.vector.tensor_add(accT[:, csl], accT[:, csl], lt[:])

            nc.sync.dma_start(y[:], accT[:])

    nc.finalize()
    return nc


def _fold_inputs(inputs):
    x = np.asarray(inputs["x"], np.float32)
    gg = np.asarray(inputs["gate_ln_g"], np.float32)
    gbeta = np.asarray(inputs["gate_ln_b"], np.float32)
    gw_ = np.asarray(inputs["gate_w"], np.float32)
    gbias = np.asarray(inputs["gate_b"], np.float32)
    eg = np.asarray(inputs["ex_ln_g"], np.float32)
    eb = np.asarray(inputs["ex_ln_b"], np.float32)
    w1_ = np.asarray(inputs["ex_w1"], np.float32)
    b1_ = np.asarray(inputs["ex_b1"], np.float32)
    w2_ = np.asarray(inputs["ex_w2"], np.float32)
    b2_ = np.asarray(inputs["ex_b2"], np.float32)

    # fold the (shared-normalize, per-head affine) LayerNorms into the
    # following linear layers: (n*g+b) @ W == n @ (g[:,None]*W) + b@W
    gwf = (gg[:, None] * gw_).astype(np.float32)                    # [D, E]
    gbf = (gbias + gbeta @ gw_).astype(np.float32)                  # [E]
    w1f = (eg[:, :, None] * w1_).astype(np.float32)                 # [E, D, H]
    b1f = (b1_ + np.einsum("ed,edh->eh", eb, w1_)).astype(np.float32)

    gw_dev = np.ascontiguousarray(gwf.reshape(KC, 128, E).transpose(1, 0, 2))
    gb_dev = np.ascontiguousarray(gbf.reshape(1, E))
    w1_dev = np.ascontiguousarray(
        w1f.reshape(E, KC, 128, H).transpose(0, 2, 1, 3))
    b1_dev = np.ascontiguousarray(b1f.reshape(E, NHC, 128).transpose(0, 2, 1))
    w2_dev = np.ascontiguousarray(
        w2_.reshape(E, NHC, 128, C).transpose(0, 2, 1, 3).reshape(
            E, 128, NHC * C))
    b2_dev = np.ascontiguousarray(b2_.T)
    weights = dict(gw=gw_dev, gb=gb_dev, w1=w1_dev, b1=b1_dev,
                   w2=w2_dev, b2=b2_dev)
    return x, weights


def _get_nc():
    if "nc" not in _NC_CACHE:
        _NC_CACHE["nc"] = _build_nc()
    return _NC_CACHE["nc"]


def _in_maps(inputs):
    x, weights = _fold_inputs(inputs)
    maps = []
    for c in range(N_CORES):
        m = dict(weights)
        m["x"] = np.ascontiguousarray(x[c * BS:(c + 1) * BS])
        maps.append(m)
    return maps


def kernel(**inputs) -> np.ndarray:
    nc = _get_nc()
    maps = _in_maps(inputs)
    res = run_bass_kernel_spmd(nc, maps, list(range(N_CORES))).results
    out = np.empty((B, C), np.float32)
    for c in range(N_CORES):
        out[c * BS:(c + 1) * BS] = res[c]["y"].T
    return out
